# revision 25
# baseline (speedup 1.0000x reference)
"""Trainium2 Bass kernel for the CKTGNN batch-of-DAGs encoder.

Strategy (pure data parallel, B=4096 split over 8 NeuronCores, 512 graphs/core):
  - one-hot inputs are built on device (iota + is_equal compares, f16); the
    per-vertex one-hot rows ride as extra K-rows inside the h-side matmul's
    third k-chunk, so the x-side contribution (incl. biases) is free,
  - the 12-step vertex scan keeps the GRU state batch-major in f16; f16 PE
    transposes (1 cyc/row) produce the feature-major copies the matmuls need,
  - the masked predecessor sum (h_in) is a running per-batch-scalar axpy over
    previously computed gated blocks (fused DVE scalar_tensor_tensor, f16),
  - a shared 109-row combo tile holds [h_in tail | one-hot | h_v tail]; the
    gh matmul reads rows 0:64, the gate matmul rows 45:109,
  - the df scatter (last-write-wins) runs on GpSimd overlapped with the scan.

kernel(**inputs) takes the full unsharded inputs, shards batch over the 8
cores, runs the SPMD bass kernel, and concatenates the shard outputs.
"""

from contextlib import ExitStack

import numpy as np

import concourse.bass as bass
import concourse.tile as tile
from concourse import bacc, mybir
from concourse.bass_utils import run_bass_kernel_spmd
from concourse.masks import make_identity

NCORES = 8
B = 4096
BL = B // NCORES          # batch per core
CH = BL // 128            # 128-row chunks per core
MAX_N = 12
NT = 10
PP = 9
HID = 301
GI = 3 * HID              # 903
OUT_W = 112

f32 = mybir.dt.float32
f16 = mybir.dt.float16
f8 = mybir.dt.float8e4
i32 = mybir.dt.int32
OP = mybir.AluOpType
AF = mybir.ActivationFunctionType

# feature-dim k-chunks of the hidden state (offset, rows); the 45-row tail
# shares a 64-row combo tile with the 19 one-hot rows
KCH = [(0, 128), (128, 128), (256, 45)]

_CACHE = {}


def _body(ctx, tc, nc, d, d_out):
    cs = lambda c: slice(128 * c, 128 * (c + 1))

    consts = ctx.enter_context(tc.tile_pool(name="consts", bufs=1))
    wp = ctx.enter_context(tc.tile_pool(name="wp", bufs=1))
    big = ctx.enter_context(tc.tile_pool(name="big", bufs=1))
    pin = ctx.enter_context(tc.tile_pool(name="pin", bufs=2))
    p_hin = ctx.enter_context(tc.tile_pool(name="p_hin", bufs=16))
    p_rz = ctx.enter_context(tc.tile_pool(name="p_rz", bufs=6))
    p_t = ctx.enter_context(tc.tile_pool(name="p_t", bufs=10))
    p_g = ctx.enter_context(tc.tile_pool(name="p_g", bufs=6))
    p_hv = ctx.enter_context(tc.tile_pool(name="p_hv", bufs=8))
    p_sg = ctx.enter_context(tc.tile_pool(name="p_sg", bufs=6))
    p_inn = ctx.enter_context(tc.tile_pool(name="p_inn", bufs=8))
    p_out = ctx.enter_context(tc.tile_pool(name="p_out", bufs=4))
    p_cmb = ctx.enter_context(tc.tile_pool(name="p_cmb", bufs=2))
    # PSUM: PB 3 x [128,1024] f32 (2 banks each) + PT 2 x [128,512] f16
    PB = ctx.enter_context(tc.tile_pool(name="PB", bufs=3, space="PSUM"))
    PT = ctx.enter_context(tc.tile_pool(name="PT", bufs=2, space="PSUM"))

    mm = nc.tensor.matmul

    # ---------------- constants ----------------
    ident = consts.tile([128, 128], f32, name="ident", tag="ident")
    make_identity(nc, ident[:])
    ident_h = consts.tile([128, 128], f16, name="ident_h", tag="ident_h")
    make_identity(nc, ident_h[:])
    cmpi = consts.tile([128, NT], i32, name="cmpi", tag="cmpi")
    nc.gpsimd.iota(cmpi[:], pattern=[[1, NT]], base=0, channel_multiplier=0)
    cmpf = consts.tile([128, NT], f32, name="cmpf", tag="cmpf")
    nc.vector.tensor_copy(cmpf[:], cmpi[:])
    zeros301b = consts.tile([128, HID], f16, name="zeros301b", tag="zeros301b")
    nc.vector.memset(zeros301b[:], 0.0)

    # ---------------- packed input DMA (4 tiles) + one-hot -----------------
    pinis = []
    qs = [nc.sync, nc.scalar, nc.gpsimd, nc.sync]
    for c in range(CH):
        t = pin.tile([128, 204], i32, name="pini", tag="pini")
        qs[c].dma_start(t[:], d["packed"][cs(c), :])
        pinis.append(t)
    # weight mega-tile, quartered across four DMA queues for bandwidth
    W = wp.tile([128, 5176], f16, name="W", tag="W")
    for q, (lo, hi) in zip([nc.scalar, nc.gpsimd, nc.sync, nc.scalar],
                           [(0, 1280), (1280, 2560), (2560, 3904),
                            (3904, 5176)]):
        q.dma_start(W[:, lo:hi], d["wpack"][:, lo:hi])
    whh = [W[:, 0:903], W[:, 903:1806], W[0:83, 1806:2709]]
    wpreI = W[:, 2709:3010]
    wgm = [W[:, 3010:3612], W[:, 3612:4214]]
    wBc = W[0:109, 4214:4816]
    wfc = [W[:, 4816:4928], W[:, 4928:5040], W[0:97, 5040:5152]]
    wdf1 = W[0:33, 5152:5168]
    wdf2 = W[0:33, 5168:5176]

    adj_f, feats, X19 = [], [], []
    for c in range(CH):
        pini = pinis[c]
        tf = pin.tile([128, MAX_N], f32, name="tf", tag="tf")
        nc.vector.tensor_copy(tf[:], pini[:, 0:12])
        pf = pin.tile([128, MAX_N], f32, name="pf", tag="pf")
        nc.vector.tensor_copy(pf[:], pini[:, 12:24])
        af = big.tile([128, MAX_N * MAX_N], f32, name=f"adj{c}", tag=f"adj{c}")
        nc.vector.tensor_copy(af[:], pini[:, 24:168])
        adj_f.append(af)
        ft = big.tile([128, 3 * MAX_N], f32, name=f"feats{c}", tag=f"feats{c}")
        nc.vector.tensor_copy(ft[:], pini[:, 168:204].bitcast(f32))
        feats.append(ft)

        x = big.tile([128, MAX_N * 19], f16, name=f"X19_{c}", tag=f"X19_{c}")
        x3 = x[:].rearrange("p (v j) -> p v j", j=19)
        nc.vector.tensor_tensor(
            x3[:, :, 0:NT],
            tf[:].unsqueeze(2).broadcast_to([128, MAX_N, NT]),
            cmpf[:, 0:NT].unsqueeze(1).broadcast_to([128, MAX_N, NT]),
            OP.is_equal,
        )
        nc.vector.tensor_tensor(
            x3[:, :, NT:19],
            pf[:].unsqueeze(2).broadcast_to([128, MAX_N, PP]),
            cmpf[:, 0:PP].unsqueeze(1).broadcast_to([128, MAX_N, PP]),
            OP.is_equal,
        )
        X19.append(x)

    # gated message store (written once per vertex, read by later vertices)
    G_all = [big.tile([128, 11 * HID], f16, name=f"G{c}", tag=f"G{c}")
             for c in range(CH)]
    # x-side candidate pre-activations, filled per step
    inn_all = [big.tile([128, MAX_N * HID], f16, name=f"inn{c}",
                        tag=f"inn{c}") for c in range(CH)]

    # feature-major h_in / h buffers (k-chunks 0,1; tails live in combo)
    hiT = [big.tile([128, BL], f16, name="hiT0", tag="hiT0"),
           big.tile([128, BL], f16, name="hiT1", tag="hiT1")]
    hvT = [big.tile([128, BL], f16, name="hvT0", tag="hvT0"),
           big.tile([128, BL], f16, name="hvT1", tag="hvT1")]
    # FC tail lhsT: rows 0:45 h-tail, 64:72 Hd, 96 ones (32-aligned writes)
    fcK2 = big.tile([97, BL], f16, name="fcK2", tag="fcK2")
    nc.vector.memset(fcK2[:], 0.0)
    nc.vector.memset(fcK2[96:97, :], 1.0)

    # df scatter accumulators (last-write-wins select chain, on GpSimd,
    # interleaved with the scan steps)
    dfs = []
    for c in range(CH):
        df = big.tile([128, 27], f32, name=f"df{c}", tag=f"df{c}")
        nc.vector.memset(df[:], 0.0)
        dfs.append(df)

    # ---------------- the vertex scan (software-pipelined) ----------------
    # step v's combo tiles + one-hot rows + inn matmuls are emitted during
    # step v-1 (between the hv transposes and the gates) so the PE queue
    # never drains at a step boundary
    pref = {}
    cmbI_d, cmbV_d = {}, {}

    def emit_prologue(v):
        # gh-side combo: rows 0:45 h_in tail, 45:64 zero pad, 64:83 one-hot
        cmbI = p_cmb.tile([83, BL], f16, name="cmbI", tag="cmbI")
        # gate-side combo: rows 0:19 one-hot, 19:64 zero pad, 64:109 h_v tail
        cmbV = p_cmb.tile([109, BL], f16, name="cmbV", tag="cmbV")
        if v < 2:  # once per pool buffer: the pad rows stay zero forever
            nc.vector.memset(cmbI[:], 0.0)
            nc.vector.memset(cmbV[:], 0.0)
        # one-hot rows: transpose X19 v-block straight into the combo tiles
        pto = PT.tile([128, BL], f16, name="ptb", tag="ptb")
        for c in range(CH):
            nc.tensor.transpose(pto[0:19, cs(c)],
                                X19[c][:, 19 * v:19 * v + 19], ident_h[:])
        nc.scalar.copy(cmbI[64:83, :], pto[0:19, :])
        nc.scalar.copy(cmbV[0:19, :], pto[0:19, :])
        # x-side candidate (inn) for this vertex: K=19 one-hot matmul;
        # two chunks share one PSUM tile (one 301-wide region per bank)
        for cc in range(0, CH, 2):
            ptI = PB.tile([128, 1024], f32, name="R", tag="R")
            for c, off in ((cc, 0), (cc + 1, 512)):
                mm(ptI[:, off:off + HID], cmbI[64:83, cs(c)],
                   wpreI[64:83, :], start=True, stop=True)
            for c, off in ((cc, 0), (cc + 1, 512)):
                nc.scalar.copy(inn_all[c][:, HID * v:HID * (v + 1)],
                               ptI[:, off:off + HID])
        cmbI_d[v], cmbV_d[v] = cmbI, cmbV

    emit_prologue(0)
    for v in range(MAX_N):
        cmbI, cmbV = cmbI_d.pop(v), cmbV_d.pop(v)

        # masked sum of predecessor gated messages: the prefix over
        # u <= v-2 was accumulated during step v-1; only the last term
        # (a[v,v-1] * G_{v-1}) lands on the critical path here
        if v > 0:
            h_ins = []
            for c in range(CH):
                a = adj_f[c]
                if v == 1:
                    hi = p_hin.tile([128, HID], f16, name="hin", tag="hin")
                    nc.vector.tensor_scalar_mul(
                        hi[:], G_all[c][:, 0:HID],
                        a[:, MAX_N * v:MAX_N * v + 1])
                else:
                    hi = pref[c]
                    u = v - 1
                    nc.vector.scalar_tensor_tensor(
                        hi[:], G_all[c][:, HID * u:HID * (u + 1)],
                        a[:, MAX_N * v + u:MAX_N * v + u + 1], hi[:],
                        op0=OP.mult, op1=OP.add)
                h_ins.append(hi)
            for k, (ko, kk) in enumerate(KCH):
                pt = PT.tile([128, BL], f16, name="ptb", tag="ptb")
                for c in range(CH):
                    nc.tensor.transpose(pt[0:kk, cs(c)],
                                        h_ins[c][:, ko:ko + kk], ident_h[:])
                if k < 2:
                    nc.scalar.copy(hiT[k][:, 0:256], pt[0:128, 0:256])
                    nc.scalar.copy(hiT[k][:, 256:512], pt[0:128, 256:512])
                else:
                    nc.scalar.copy(cmbI[0:45, 0:256], pt[0:45, 0:256])
                    nc.scalar.copy(cmbI[0:45, 256:512], pt[0:45, 256:512])
        else:
            h_ins = [zeros301b] * CH

        # prefix of step v+1's masked sum (terms u <= v-1) as an op list;
        # the first few fill the R-matmul latency before the GRU, the rest
        # fill the gate-matmul wait before the G multiply
        pref_ops = []
        if 1 <= v < MAX_N - 1:
            for c in range(CH):
                w = v + 1
                a = adj_f[c]
                p = p_hin.tile([128, HID], f16, name="hin", tag="hin")
                pref[c] = p
                def op0_(c=c, p=p, a=a, w=w):
                    nc.vector.tensor_scalar_mul(
                        p[:], G_all[c][:, 0:HID],
                        a[:, MAX_N * w:MAX_N * w + 1])
                pref_ops.append(op0_)
                for u in range(1, v):
                    def opu_(c=c, p=p, a=a, w=w, u=u):
                        nc.vector.scalar_tensor_tensor(
                            p[:], G_all[c][:, HID * u:HID * (u + 1)],
                            a[:, MAX_N * w + u:MAX_N * w + u + 1], p[:],
                            op0=OP.mult, op1=OP.add)
                    pref_ops.append(opu_)
        for f in pref_ops[:8]:
            f()

        # gi+gh fused in PSUM, then the GRU cell elementwise
        hvs = []
        for c in range(CH):
            R = PB.tile([128, 1024], f32, name="R", tag="R")
            if v > 0:
                for k in range(2):
                    l = hiT[k][:, cs(c)]
                    mm(R[:, 0:512], l, whh[k][:, 0:512],
                       start=(k == 0), stop=False)
                    mm(R[:, 512:903], l, whh[k][:, 512:903],
                       start=(k == 0), stop=False)
                l2 = cmbI[0:83, cs(c)]
                mm(R[:, 0:512], l2, whh[2][:, 0:512], start=False, stop=True)
                mm(R[:, 512:903], l2, whh[2][:, 512:903],
                   start=False, stop=True)
            else:
                l2 = cmbI[0:83, cs(c)]
                mm(R[:, 0:512], l2, whh[2][:, 0:512], start=True, stop=True)
                mm(R[:, 512:903], l2, whh[2][:, 512:903],
                   start=True, stop=True)
            rz = p_rz.tile([128, 602], f16, name="rz", tag="rz")
            nc.scalar.activation(rz[:], R[:, 0:602], AF.Sigmoid)
            tn = p_t.tile([128, HID], f16, name="tn", tag="tn")
            nc.vector.tensor_mul(tn[:], rz[:, 0:HID], R[:, 602:903])
            tn2 = p_t.tile([128, HID], f16, name="tn2", tag="tn2")
            nc.vector.tensor_add(tn2[:], tn[:],
                                 inn_all[c][:, HID * v:HID * (v + 1)])
            g = p_g.tile([128, HID], f16, name="g", tag="g")
            nc.scalar.activation(g[:], tn2[:], AF.Tanh)
            t3 = p_t.tile([128, HID], f16, name="t3", tag="t3")
            nc.vector.tensor_sub(t3[:], h_ins[c][:], g[:])
            t4 = p_t.tile([128, HID], f16, name="t4", tag="t4")
            nc.vector.tensor_mul(t4[:], t3[:], rz[:, HID:602])
            hv = p_hv.tile([128, HID], f16, name="hv", tag="hv")
            nc.vector.tensor_add(hv[:], g[:], t4[:])
            hvs.append(hv)

        # transpose h_v to feature-major (step 11 k=2 lands in the FC tile);
        # k=2 first: it feeds the gates' combo matmul
        for k, (ko, kk) in [(2, KCH[2]), (0, KCH[0]), (1, KCH[1])]:
            pt = PT.tile([128, BL], f16, name="ptb", tag="ptb")
            for c in range(CH):
                nc.tensor.transpose(pt[0:kk, cs(c)],
                                    hvs[c][:, ko:ko + kk], ident_h[:])
            if k < 2:
                nc.scalar.copy(hvT[k][:, 0:256], pt[0:128, 0:256])
                nc.scalar.copy(hvT[k][:, 256:512], pt[0:128, 256:512])
            elif v == MAX_N - 1:
                nc.scalar.copy(fcK2[0:45, :], pt[0:45, :])
            else:
                nc.scalar.copy(cmbV[64:109, 0:256], pt[0:45, 0:256])
                nc.scalar.copy(cmbV[64:109, 256:512], pt[0:45, 256:512])

        # next step's combo/one-hot/inn: fills the PE while the hvT copies
        # land before the gates
        if v + 1 < MAX_N:
            emit_prologue(v + 1)


        for f in pref_ops[8:]:
            f()

        # df select chain, two vertices per step: fills the DVE wait for the
        # first gate matmuls (on DVE for early steps, GpSimd later)
        deng = nc.vector if v < 4 else nc.gpsimd
        for dv in ([2 * v, 2 * v + 1] if v < 6 else []):
            for c in range(CH):
                df3 = dfs[c][:].rearrange("p (q j) -> p q j", j=3)
                f3 = feats[c][:, 3 * dv:3 * dv + 3].unsqueeze(1) \
                    .broadcast_to([128, PP, 3])
                oh = X19[c][:, 19 * dv + NT:19 * dv + 19].unsqueeze(2) \
                    .broadcast_to([128, PP, 3])
                s = p_t.tile([128, 27], f32, name="dfs", tag="dfs")
                s3 = s[:].rearrange("p (q j) -> p q j", j=3)
                deng.tensor_sub(s3, f3, df3)
                w2 = p_t.tile([128, 27], f32, name="dfw", tag="dfw")
                w3 = w2[:].rearrange("p (q j) -> p q j", j=3)
                deng.tensor_mul(w3, oh, s3)
                deng.tensor_add(df3, df3, w3)

        # gate * mapper on [h_v, pos-onehot]; cmbV rows carry the one-hot
        # (pos+bg via wBc109) and the h_v tail
        if v < MAX_N - 1:
            for c in range(CH):
                P = PB.tile([128, 1024], f32, name="R", tag="R")
                cv = cmbV[0:109, cs(c)]
                mm(P[:, 0:512], cv, wBc[:, 0:512], start=True, stop=False)
                mm(P[:, 512:602], cv, wBc[:, 512:602], start=True, stop=False)
                for k in range(2):
                    l = hvT[k][:, cs(c)]
                    last = k == 1
                    mm(P[:, 0:512], l, wgm[k][:, 0:512],
                       start=False, stop=last)
                    mm(P[:, 512:602], l, wgm[k][:, 512:602],
                       start=False, stop=last)
                sg = p_sg.tile([128, HID], f16, name="sg", tag="sg")
                nc.scalar.activation(sg[:], P[:, 0:HID], AF.Sigmoid)
                nc.vector.tensor_mul(G_all[c][:, HID * v:HID * (v + 1)],
                                     sg[:], P[:, HID:602])

        # df select chain, two vertices per step on GpSimd: done by step 5,
        # so the df MLP can run mid-scan instead of as a tail

        if v == 7:
            # df MLP mid-scan (scatter drained by step 5)
            dfT = big.tile([33, BL], f16, name="dfT", tag="dfT")
            nc.vector.memset(dfT[:], 0.0)
            nc.vector.memset(dfT[32:33, :], 1.0)
            for c in range(CH):
                dfb = p_t.tile([128, 27], f16, name="dfb", tag="dfb")
                nc.vector.tensor_copy(dfb[:], dfs[c][:])
                pt = PT.tile([128, BL], f16, name="ptb", tag="ptb")
                nc.tensor.transpose(pt[0:27, cs(c)], dfb[:], ident_h[:])
                nc.any.tensor_copy(dfT[0:27, cs(c)], pt[0:27, cs(c)])
            pd1 = PB.tile([128, 1024], f32, name="R", tag="R")
            mm(pd1[0:16, 0:BL], wdf1[0:33, :], dfT[:], start=True, stop=True)
            r1T = big.tile([33, BL], f16, name="r1T", tag="r1T")
            nc.vector.memset(r1T[:], 0.0)
            nc.vector.memset(r1T[32:33, :], 1.0)
            nc.scalar.activation(r1T[0:16, :], pd1[0:16, 0:BL], AF.Relu)
            pd2 = PB.tile([128, 1024], f32, name="R", tag="R")
            mm(pd2[0:8, 0:BL], wdf2[0:33, :], r1T[:], start=True, stop=True)
            nc.any.tensor_copy(fcK2[64:72, :], pd2[0:8, 0:BL])

    # ---------------- final fully-connected (mu | logvar) ----------------
    for c in range(CH):
        po = PB.tile([128, 1024], f32, name="R", tag="R")
        mm(po[:, 0:OUT_W], hvT[0][:, cs(c)], wfc[0], start=True, stop=False)
        mm(po[:, 0:OUT_W], hvT[1][:, cs(c)], wfc[1], start=False, stop=False)
        mm(po[:, 0:OUT_W], fcK2[0:97, cs(c)], wfc[2], start=False, stop=True)
        ob = p_out.tile([128, OUT_W], f32, name="ob", tag="ob")
        nc.any.tensor_copy(ob[:], po[:, 0:OUT_W])
        nc.sync.dma_start(d_out[cs(c), :], ob[:])


def build_nc():
    nc = bacc.Bacc("TRN2", target_bir_lowering=False, debug=False,
                   num_devices=NCORES)
    d = {}
    for name, shape, dt in [
        ("packed", [BL, 204], i32),
        ("wpack", [128, 5176], f16),
    ]:
        d[name] = nc.dram_tensor(name, shape, dt, kind="ExternalInput").ap()
    d_out = nc.dram_tensor("out", [BL, OUT_W], f32, kind="ExternalOutput").ap()
    with tile.TileContext(nc) as tc:
        with ExitStack() as ctx:
            _body(ctx, tc, nc, d, d_out)
    nc.compile()
    return nc


def prepack(inputs):
    ii = {k: np.asarray(v) for k, v in inputs.items()}
    W_ih, b_ih = ii["W_ih"].astype(np.float32), ii["b_ih"].astype(np.float32)
    Wg, bg = ii["Wg"].astype(np.float32), ii["bg"].astype(np.float32)
    Wm = ii["Wm"].astype(np.float32)
    b_hh = ii["b_hh"].astype(np.float32)
    # one-hot-row weights: gi r/z parts (+b_ih+b_hh) in cols [0:602]; cols
    # [602:903] carry only b_hh's candidate part (inn itself is computed
    # separately via wpreI)
    wpreA = W_ih.T.copy()
    wpreA[:, 602:903] = 0.0
    wpreA[:NT, 0:602] += (b_ih + b_hh)[None, 0:602]
    wpreA[:NT, 602:903] += b_hh[None, 602:903]
    wpreI = W_ih.T[:, 602:903].copy()
    wpreI[:NT] += b_ih[None, 602:903]
    wpreB = np.zeros((19, 602), np.float32)
    wpreB[NT:19, 0:HID] = Wg[:, HID:HID + PP].T + bg[None, :]
    wpreB[NT:19, HID:602] = Wm[:, HID:HID + PP].T
    whhT = ii["W_hh"].astype(np.float32).T.copy()       # [301, 903]
    wgm = np.zeros((HID, 602), np.float32)
    wgm[:, 0:HID] = Wg[:, 0:HID].T
    wgm[:, HID:602] = Wm[:, 0:HID].T
    # h-side k2: rows 0:45 = W_hh^T tail, 45:64 zero pad, 64:83 one-hot
    whh2 = np.zeros((83, GI), np.float32)
    whh2[0:45] = whhT[256:301]
    whh2[64:83] = wpreA
    # gates combo: rows 0:19 = one-hot (pos+bg), rows 64:109 = wgm tail
    wBc109 = np.zeros((109, 602), np.float32)
    wBc109[0:19] = wpreB
    wBc109[64:109] = wgm[256:301]
    # FC lhsT rows: [0:256) = h dims 0:256 (two 128-chunks); tail chunk of 97
    # rows: 0:45 h-tail, 64:72 Hd, 96 biases (matches fcK2 on-device layout)
    wfcT1 = ii["W_fc1"].astype(np.float32).T   # [309, 56]
    wfcT2 = ii["W_fc2"].astype(np.float32).T
    wfc = np.zeros((353, OUT_W), np.float32)
    wfc[0:256, 0:56] = wfcT1[0:256]
    wfc[0:256, 56:112] = wfcT2[0:256]
    tail = np.zeros((97, OUT_W), np.float32)
    tail[0:45, 0:56] = wfcT1[256:301]
    tail[0:45, 56:112] = wfcT2[256:301]
    tail[64:72, 0:56] = wfcT1[301:309]
    tail[64:72, 56:112] = wfcT2[301:309]
    tail[96, 0:56] = ii["b_fc1"].astype(np.float32)
    tail[96, 56:112] = ii["b_fc2"].astype(np.float32)
    wfc[256:353] = tail
    wdf1 = np.zeros((33, 16), np.float32)
    wdf1[0:27] = ii["W_df1"].astype(np.float32).T
    wdf1[32] = ii["b_df1"].astype(np.float32)
    wdf2 = np.zeros((33, 8), np.float32)
    wdf2[0:16] = ii["W_df2"].astype(np.float32).T
    wdf2[32] = ii["b_df2"].astype(np.float32)

    # one mega weight block [128, 5176]
    wpk = np.zeros((128, 5176), np.float32)
    wpk[:, 0:903] = whhT[0:128]
    wpk[:, 903:1806] = whhT[128:256]
    wpk[0:83, 1806:2709] = whh2
    wpk[64:83, 2709:3010] = wpreI
    wpk[:, 3010:3612] = wgm[0:128]
    wpk[:, 3612:4214] = wgm[128:256]
    wpk[0:109, 4214:4816] = wBc109
    wpk[:, 4816:4928] = wfc[0:128]
    wpk[:, 4928:5040] = wfc[128:256]
    wpk[0:97, 5040:5152] = wfc[256:353]
    wpk[0:33, 5152:5168] = wdf1
    wpk[0:33, 5168:5176] = wdf2
    return {"wpack": np.ascontiguousarray(wpk).astype(np.float16)}


def shard_inputs(inputs):
    ii = {k: np.asarray(v) for k, v in inputs.items()}
    w = prepack(ii)
    types = ii["types"].astype(np.int32)
    paths = ii["paths"].astype(np.int32)
    adj = ii["adj_raw"].reshape(B, MAX_N * MAX_N).astype(np.int32)
    feats = ii["feats"].reshape(B, 3 * MAX_N).astype(np.float32).view(np.int32)
    packed = np.concatenate([types, paths, adj, feats], axis=1)
    maps = []
    for i in range(NCORES):
        sl = slice(i * BL, (i + 1) * BL)
        m = dict(packed=np.ascontiguousarray(packed[sl]), **w)
        maps.append(m)
    return maps


def get_nc():
    if "nc" not in _CACHE:
        _CACHE["nc"] = build_nc()
    return _CACHE["nc"]


def kernel(**inputs):
    nc = get_nc()
    maps = shard_inputs(inputs)
    res = run_bass_kernel_spmd(nc, maps, list(range(NCORES)))
    out = np.concatenate([res.results[i]["out"] for i in range(NCORES)], axis=0)
    return np.ascontiguousarray(out.astype(np.float32))


# revision 27
# speedup vs baseline: 1.0396x; 1.0396x over previous
"""Trainium2 Bass kernel for the CKTGNN batch-of-DAGs encoder.

Strategy (pure data parallel, B=4096 split over 8 NeuronCores, 512 graphs/core):
  - one-hot inputs are built on device (iota + is_equal compares, f16); the
    per-vertex one-hot rows ride as extra K-rows inside the h-side matmul's
    third k-chunk, so the x-side contribution (incl. biases) is free,
  - the 12-step vertex scan keeps the GRU state batch-major in f16; f16 PE
    transposes (1 cyc/row) produce the feature-major copies the matmuls need,
  - the masked predecessor sum (h_in) is a running per-batch-scalar axpy over
    previously computed gated blocks (fused DVE scalar_tensor_tensor, f16),
  - a shared 109-row combo tile holds [h_in tail | one-hot | h_v tail]; the
    gh matmul reads rows 0:64, the gate matmul rows 45:109,
  - the df scatter (last-write-wins) runs on GpSimd overlapped with the scan.

kernel(**inputs) takes the full unsharded inputs, shards batch over the 8
cores, runs the SPMD bass kernel, and concatenates the shard outputs.
"""

from contextlib import ExitStack

import numpy as np

import concourse.bass as bass
import concourse.tile as tile
from concourse import bacc, mybir
from concourse.bass_utils import run_bass_kernel_spmd
from concourse.masks import make_identity

NCORES = 8
B = 4096
BL = B // NCORES          # batch per core
CH = BL // 128            # 128-row chunks per core
MAX_N = 12
NT = 10
PP = 9
HID = 301
GI = 3 * HID              # 903
OUT_W = 112

f32 = mybir.dt.float32
f16 = mybir.dt.float16
f8 = mybir.dt.float8e4
i32 = mybir.dt.int32
OP = mybir.AluOpType
AF = mybir.ActivationFunctionType

# feature-dim k-chunks of the hidden state (offset, rows); the 45-row tail
# shares a 64-row combo tile with the 19 one-hot rows
KCH = [(0, 128), (128, 128), (256, 45)]

_CACHE = {}


def _body(ctx, tc, nc, d, d_out):
    cs = lambda c: slice(128 * c, 128 * (c + 1))

    consts = ctx.enter_context(tc.tile_pool(name="consts", bufs=1))
    wp = ctx.enter_context(tc.tile_pool(name="wp", bufs=1))
    big = ctx.enter_context(tc.tile_pool(name="big", bufs=1))
    pin = ctx.enter_context(tc.tile_pool(name="pin", bufs=2))
    p_hin = ctx.enter_context(tc.tile_pool(name="p_hin", bufs=16))
    p_rz = ctx.enter_context(tc.tile_pool(name="p_rz", bufs=6))
    p_t = ctx.enter_context(tc.tile_pool(name="p_t", bufs=10))
    p_g = ctx.enter_context(tc.tile_pool(name="p_g", bufs=6))
    p_hv = ctx.enter_context(tc.tile_pool(name="p_hv", bufs=8))
    p_sg = ctx.enter_context(tc.tile_pool(name="p_sg", bufs=6))
    p_inn = ctx.enter_context(tc.tile_pool(name="p_inn", bufs=8))
    p_out = ctx.enter_context(tc.tile_pool(name="p_out", bufs=4))
    p_cmb = ctx.enter_context(tc.tile_pool(name="p_cmb", bufs=2))
    # PSUM: PB 3 x [128,1024] f32 (2 banks each) + PT 2 x [128,512] f16
    PB = ctx.enter_context(tc.tile_pool(name="PB", bufs=3, space="PSUM"))
    PT = ctx.enter_context(tc.tile_pool(name="PT", bufs=2, space="PSUM"))

    mm = nc.tensor.matmul

    # ---------------- constants ----------------
    ident = consts.tile([128, 128], f32, name="ident", tag="ident")
    make_identity(nc, ident[:])
    ident_h = consts.tile([128, 128], f16, name="ident_h", tag="ident_h")
    make_identity(nc, ident_h[:])
    cmpi = consts.tile([128, NT], i32, name="cmpi", tag="cmpi")
    nc.gpsimd.iota(cmpi[:], pattern=[[1, NT]], base=0, channel_multiplier=0)
    cmpf = consts.tile([128, NT], f32, name="cmpf", tag="cmpf")
    nc.vector.tensor_copy(cmpf[:], cmpi[:])
    zeros301b = consts.tile([128, HID], f16, name="zeros301b", tag="zeros301b")
    nc.vector.memset(zeros301b[:], 0.0)

    # ---------------- packed input DMA (4 tiles) + one-hot -----------------
    pinis = []
    qs = [nc.sync, nc.scalar, nc.gpsimd, nc.sync]
    for c in range(CH):
        t = pin.tile([128, 204], i32, name="pini", tag="pini")
        qs[c].dma_start(t[:], d["packed"][cs(c), :])
        pinis.append(t)
    # weight mega-tile, quartered across four DMA queues for bandwidth
    W = wp.tile([128, 5176], f16, name="W", tag="W")
    for q, (lo, hi) in zip([nc.scalar, nc.gpsimd, nc.sync, nc.scalar],
                           [(0, 1280), (1280, 2560), (2560, 3904),
                            (3904, 5176)]):
        q.dma_start(W[:, lo:hi], d["wpack"][:, lo:hi])
    whh = [W[:, 0:903], W[:, 903:1806], W[0:83, 1806:2709]]
    wpreI = W[:, 2709:3010]
    wgm = [W[:, 3010:3612], W[:, 3612:4214]]
    wBc = W[0:109, 4214:4816]
    wfc = [W[:, 4816:4928], W[:, 4928:5040], W[0:97, 5040:5152]]
    wdf1 = W[0:33, 5152:5168]
    wdf2 = W[0:33, 5168:5176]

    adj_f, feats, X19 = [], [], []
    for c in range(CH):
        pini = pinis[c]
        tf = pin.tile([128, MAX_N], f32, name="tf", tag="tf")
        nc.vector.tensor_copy(tf[:], pini[:, 0:12])
        pf = pin.tile([128, MAX_N], f32, name="pf", tag="pf")
        nc.vector.tensor_copy(pf[:], pini[:, 12:24])
        af = big.tile([128, MAX_N * MAX_N], f32, name=f"adj{c}", tag=f"adj{c}")
        nc.vector.tensor_copy(af[:], pini[:, 24:168])
        adj_f.append(af)
        ft = big.tile([128, 3 * MAX_N], f32, name=f"feats{c}", tag=f"feats{c}")
        nc.vector.tensor_copy(ft[:], pini[:, 168:204].bitcast(f32))
        feats.append(ft)

        x = big.tile([128, MAX_N * 19], f16, name=f"X19_{c}", tag=f"X19_{c}")
        x3 = x[:].rearrange("p (v j) -> p v j", j=19)
        nc.vector.tensor_tensor(
            x3[:, :, 0:NT],
            tf[:].unsqueeze(2).broadcast_to([128, MAX_N, NT]),
            cmpf[:, 0:NT].unsqueeze(1).broadcast_to([128, MAX_N, NT]),
            OP.is_equal,
        )
        nc.vector.tensor_tensor(
            x3[:, :, NT:19],
            pf[:].unsqueeze(2).broadcast_to([128, MAX_N, PP]),
            cmpf[:, 0:PP].unsqueeze(1).broadcast_to([128, MAX_N, PP]),
            OP.is_equal,
        )
        X19.append(x)

    # all 12 one-hot v-blocks transposed once into SBUF staging tiles
    # (4 blocks per tile at 32-aligned partition offsets)
    XvTs = [big.tile([128, BL], f16, name=f"XvT{q}", tag=f"XvT{q}")
            for q in range(3)]
    for q in range(3):
        for j in range(4):
            vv = 4 * q + j
            pto = PT.tile([128, BL], f16, name="ptb", tag="ptb")
            for c in range(CH):
                nc.tensor.transpose(pto[0:19, cs(c)],
                                    X19[c][:, 19 * vv:19 * vv + 19],
                                    ident_h[:])
            nc.scalar.copy(XvTs[q][32 * j:32 * j + 19, :], pto[0:19, :])

    # gated message store (written once per vertex, read by later vertices)
    G_all = [big.tile([128, 11 * HID], f16, name=f"G{c}", tag=f"G{c}")
             for c in range(CH)]
    # x-side candidate pre-activations, filled per step
    inn_all = [big.tile([128, MAX_N * HID], f16, name=f"inn{c}",
                        tag=f"inn{c}") for c in range(CH)]

    # feature-major h_in / h buffers (k-chunks 0,1; tails live in combo)
    hiT = [big.tile([128, BL], f16, name="hiT0", tag="hiT0"),
           big.tile([128, BL], f16, name="hiT1", tag="hiT1")]
    hvT = [big.tile([128, BL], f16, name="hvT0", tag="hvT0"),
           big.tile([128, BL], f16, name="hvT1", tag="hvT1")]
    # FC tail lhsT: rows 0:45 h-tail, 64:72 Hd, 96 ones (32-aligned writes)
    fcK2 = big.tile([97, BL], f16, name="fcK2", tag="fcK2")
    nc.vector.memset(fcK2[:], 0.0)
    nc.vector.memset(fcK2[96:97, :], 1.0)

    # df scatter accumulators (last-write-wins select chain, on GpSimd,
    # interleaved with the scan steps)
    dfs = []
    for c in range(CH):
        df = big.tile([128, 27], f32, name=f"df{c}", tag=f"df{c}")
        nc.vector.memset(df[:], 0.0)
        dfs.append(df)

    # ---------------- the vertex scan (software-pipelined) ----------------
    # step v's combo tiles + one-hot rows + inn matmuls are emitted during
    # step v-1 (between the hv transposes and the gates) so the PE queue
    # never drains at a step boundary
    pref = {}
    cmbI_d, cmbV_d = {}, {}

    def emit_prologue(v):
        # gh-side combo: rows 0:45 h_in tail, 45:64 zero pad, 64:83 one-hot
        cmbI = p_cmb.tile([83, BL], f16, name="cmbI", tag="cmbI")
        # gate-side combo: rows 0:19 one-hot, 19:64 zero pad, 64:109 h_v tail
        cmbV = p_cmb.tile([109, BL], f16, name="cmbV", tag="cmbV")
        if v < 2:  # once per pool buffer: the pad rows stay zero forever
            nc.vector.memset(cmbI[:], 0.0)
            nc.vector.memset(cmbV[:], 0.0)
        # one-hot rows from the preamble staging store
        xs = XvTs[v // 4][32 * (v % 4):32 * (v % 4) + 19, :]
        nc.scalar.copy(cmbI[64:83, :], xs)
        nc.scalar.copy(cmbV[0:19, :], xs)
        # x-side candidate (inn) for this vertex: K=19 one-hot matmul;
        # two chunks share one PSUM tile (one 301-wide region per bank)
        for cc in range(0, CH, 2):
            ptI = PB.tile([128, 1024], f32, name="R", tag="R")
            for c, off in ((cc, 0), (cc + 1, 512)):
                mm(ptI[:, off:off + HID], cmbI[64:83, cs(c)],
                   wpreI[64:83, :], start=True, stop=True)
            for c, off in ((cc, 0), (cc + 1, 512)):
                nc.scalar.copy(inn_all[c][:, HID * v:HID * (v + 1)],
                               ptI[:, off:off + HID])
        cmbI_d[v], cmbV_d[v] = cmbI, cmbV

    emit_prologue(0)
    for v in range(MAX_N):
        cmbI, cmbV = cmbI_d.pop(v), cmbV_d.pop(v)

        # masked sum of predecessor gated messages: the prefix over
        # u <= v-2 was accumulated during step v-1; only the last term
        # (a[v,v-1] * G_{v-1}) lands on the critical path here
        if v > 0:
            h_ins = []
            for c in range(CH):
                a = adj_f[c]
                if v == 1:
                    hi = p_hin.tile([128, HID], f16, name="hin", tag="hin")
                    nc.vector.tensor_scalar_mul(
                        hi[:], G_all[c][:, 0:HID],
                        a[:, MAX_N * v:MAX_N * v + 1])
                else:
                    hi = pref[c]
                    u = v - 1
                    nc.vector.scalar_tensor_tensor(
                        hi[:], G_all[c][:, HID * u:HID * (u + 1)],
                        a[:, MAX_N * v + u:MAX_N * v + u + 1], hi[:],
                        op0=OP.mult, op1=OP.add)
                h_ins.append(hi)
            for k, (ko, kk) in enumerate(KCH):
                pt = PT.tile([128, BL], f16, name="ptb", tag="ptb")
                for c in range(CH):
                    nc.tensor.transpose(pt[0:kk, cs(c)],
                                        h_ins[c][:, ko:ko + kk], ident_h[:])
                if k < 2:
                    nc.scalar.copy(hiT[k][:], pt[0:128, :])
                else:
                    nc.scalar.copy(cmbI[0:45, :], pt[0:45, :])
        else:
            h_ins = [zeros301b] * CH

        # prefix of step v+1's masked sum (terms u <= v-1) as an op list;
        # the first few fill the R-matmul latency before the GRU, the rest
        # fill the gate-matmul wait before the G multiply
        pref_ops = []
        if 1 <= v < MAX_N - 1:
            for c in range(CH):
                w = v + 1
                a = adj_f[c]
                p = p_hin.tile([128, HID], f16, name="hin", tag="hin")
                pref[c] = p
                def op0_(c=c, p=p, a=a, w=w):
                    nc.vector.tensor_scalar_mul(
                        p[:], G_all[c][:, 0:HID],
                        a[:, MAX_N * w:MAX_N * w + 1])
                pref_ops.append(op0_)
                for u in range(1, v):
                    def opu_(c=c, p=p, a=a, w=w, u=u):
                        nc.vector.scalar_tensor_tensor(
                            p[:], G_all[c][:, HID * u:HID * (u + 1)],
                            a[:, MAX_N * w + u:MAX_N * w + u + 1], p[:],
                            op0=OP.mult, op1=OP.add)
                    pref_ops.append(opu_)
        for f in pref_ops[:8]:
            f()

        # gi+gh fused in PSUM, then the GRU cell elementwise
        hvs = []
        for c in range(CH):
            R = PB.tile([128, 1024], f32, name="R", tag="R")
            if v > 0:
                for k in range(2):
                    l = hiT[k][:, cs(c)]
                    mm(R[:, 0:512], l, whh[k][:, 0:512],
                       start=(k == 0), stop=False)
                    mm(R[:, 512:903], l, whh[k][:, 512:903],
                       start=(k == 0), stop=False)
                l2 = cmbI[0:83, cs(c)]
                mm(R[:, 0:512], l2, whh[2][:, 0:512], start=False, stop=True)
                mm(R[:, 512:903], l2, whh[2][:, 512:903],
                   start=False, stop=True)
            else:
                l2 = cmbI[0:83, cs(c)]
                mm(R[:, 0:512], l2, whh[2][:, 0:512], start=True, stop=True)
                mm(R[:, 512:903], l2, whh[2][:, 512:903],
                   start=True, stop=True)
            rz = p_rz.tile([128, 602], f16, name="rz", tag="rz")
            nc.scalar.activation(rz[:], R[:, 0:602], AF.Sigmoid)
            tn = p_t.tile([128, HID], f16, name="tn", tag="tn")
            nc.vector.tensor_mul(tn[:], rz[:, 0:HID], R[:, 602:903])
            tn2 = p_t.tile([128, HID], f16, name="tn2", tag="tn2")
            nc.vector.tensor_add(tn2[:], tn[:],
                                 inn_all[c][:, HID * v:HID * (v + 1)])
            g = p_g.tile([128, HID], f16, name="g", tag="g")
            nc.scalar.activation(g[:], tn2[:], AF.Tanh)
            t3 = p_t.tile([128, HID], f16, name="t3", tag="t3")
            nc.vector.tensor_sub(t3[:], h_ins[c][:], g[:])
            t4 = p_t.tile([128, HID], f16, name="t4", tag="t4")
            nc.vector.tensor_mul(t4[:], t3[:], rz[:, HID:602])
            hv = p_hv.tile([128, HID], f16, name="hv", tag="hv")
            nc.vector.tensor_add(hv[:], g[:], t4[:])
            hvs.append(hv)

        # transpose h_v to feature-major (step 11 k=2 lands in the FC tile);
        # k=2 first: it feeds the gates' combo matmul
        for k, (ko, kk) in [(2, KCH[2]), (0, KCH[0]), (1, KCH[1])]:
            pt = PT.tile([128, BL], f16, name="ptb", tag="ptb")
            for c in range(CH):
                nc.tensor.transpose(pt[0:kk, cs(c)],
                                    hvs[c][:, ko:ko + kk], ident_h[:])
            if k < 2:
                nc.scalar.copy(hvT[k][:], pt[0:128, :])
            elif v == MAX_N - 1:
                nc.scalar.copy(fcK2[0:45, :], pt[0:45, :])
            else:
                nc.scalar.copy(cmbV[64:109, :], pt[0:45, :])

        # next step's combo/one-hot/inn: fills the PE while the hvT copies
        # land before the gates
        if v + 1 < MAX_N:
            emit_prologue(v + 1)


        for f in pref_ops[8:]:
            f()

        # df select chain, two vertices per step: fills the DVE wait for the
        # first gate matmuls (on DVE for early steps, GpSimd later)
        deng = nc.vector if v < 4 else nc.gpsimd
        for dv in ([2 * v, 2 * v + 1] if v < 6 else []):
            for c in range(CH):
                df3 = dfs[c][:].rearrange("p (q j) -> p q j", j=3)
                f3 = feats[c][:, 3 * dv:3 * dv + 3].unsqueeze(1) \
                    .broadcast_to([128, PP, 3])
                oh = X19[c][:, 19 * dv + NT:19 * dv + 19].unsqueeze(2) \
                    .broadcast_to([128, PP, 3])
                s = p_t.tile([128, 27], f32, name="dfs", tag="dfs")
                s3 = s[:].rearrange("p (q j) -> p q j", j=3)
                deng.tensor_sub(s3, f3, df3)
                w2 = p_t.tile([128, 27], f32, name="dfw", tag="dfw")
                w3 = w2[:].rearrange("p (q j) -> p q j", j=3)
                deng.tensor_mul(w3, oh, s3)
                deng.tensor_add(df3, df3, w3)

        # gate * mapper on [h_v, pos-onehot]; cmbV rows carry the one-hot
        # (pos+bg via wBc109) and the h_v tail
        if v < MAX_N - 1:
            for c in range(CH):
                P = PB.tile([128, 1024], f32, name="R", tag="R")
                cv = cmbV[0:109, cs(c)]
                mm(P[:, 0:512], cv, wBc[:, 0:512], start=True, stop=False)
                mm(P[:, 512:602], cv, wBc[:, 512:602], start=True, stop=False)
                for k in range(2):
                    l = hvT[k][:, cs(c)]
                    last = k == 1
                    mm(P[:, 0:512], l, wgm[k][:, 0:512],
                       start=False, stop=last)
                    mm(P[:, 512:602], l, wgm[k][:, 512:602],
                       start=False, stop=last)
                sg = p_sg.tile([128, HID], f16, name="sg", tag="sg")
                nc.scalar.activation(sg[:], P[:, 0:HID], AF.Sigmoid)
                nc.vector.tensor_mul(G_all[c][:, HID * v:HID * (v + 1)],
                                     sg[:], P[:, HID:602])

        # df select chain, two vertices per step on GpSimd: done by step 5,
        # so the df MLP can run mid-scan instead of as a tail

        if v == 7:
            # df MLP mid-scan (scatter drained by step 5)
            dfT = big.tile([33, BL], f16, name="dfT", tag="dfT")
            nc.vector.memset(dfT[:], 0.0)
            nc.vector.memset(dfT[32:33, :], 1.0)
            for c in range(CH):
                dfb = p_t.tile([128, 27], f16, name="dfb", tag="dfb")
                nc.vector.tensor_copy(dfb[:], dfs[c][:])
                pt = PT.tile([128, BL], f16, name="ptb", tag="ptb")
                nc.tensor.transpose(pt[0:27, cs(c)], dfb[:], ident_h[:])
                nc.any.tensor_copy(dfT[0:27, cs(c)], pt[0:27, cs(c)])
            pd1 = PB.tile([128, 1024], f32, name="R", tag="R")
            mm(pd1[0:16, 0:BL], wdf1[0:33, :], dfT[:], start=True, stop=True)
            r1T = big.tile([33, BL], f16, name="r1T", tag="r1T")
            nc.vector.memset(r1T[:], 0.0)
            nc.vector.memset(r1T[32:33, :], 1.0)
            nc.scalar.activation(r1T[0:16, :], pd1[0:16, 0:BL], AF.Relu)
            pd2 = PB.tile([128, 1024], f32, name="R", tag="R")
            mm(pd2[0:8, 0:BL], wdf2[0:33, :], r1T[:], start=True, stop=True)
            nc.any.tensor_copy(fcK2[64:72, :], pd2[0:8, 0:BL])

    # ---------------- final fully-connected (mu | logvar) ----------------
    for c in range(CH):
        po = PB.tile([128, 1024], f32, name="R", tag="R")
        mm(po[:, 0:OUT_W], hvT[0][:, cs(c)], wfc[0], start=True, stop=False)
        mm(po[:, 0:OUT_W], hvT[1][:, cs(c)], wfc[1], start=False, stop=False)
        mm(po[:, 0:OUT_W], fcK2[0:97, cs(c)], wfc[2], start=False, stop=True)
        ob = p_out.tile([128, OUT_W], f32, name="ob", tag="ob")
        nc.any.tensor_copy(ob[:], po[:, 0:OUT_W])
        nc.sync.dma_start(d_out[cs(c), :], ob[:])


def build_nc():
    nc = bacc.Bacc("TRN2", target_bir_lowering=False, debug=False,
                   num_devices=NCORES)
    d = {}
    for name, shape, dt in [
        ("packed", [BL, 204], i32),
        ("wpack", [128, 5176], f16),
    ]:
        d[name] = nc.dram_tensor(name, shape, dt, kind="ExternalInput").ap()
    d_out = nc.dram_tensor("out", [BL, OUT_W], f32, kind="ExternalOutput").ap()
    with tile.TileContext(nc) as tc:
        with ExitStack() as ctx:
            _body(ctx, tc, nc, d, d_out)
    nc.compile()
    return nc


def prepack(inputs):
    ii = {k: np.asarray(v) for k, v in inputs.items()}
    W_ih, b_ih = ii["W_ih"].astype(np.float32), ii["b_ih"].astype(np.float32)
    Wg, bg = ii["Wg"].astype(np.float32), ii["bg"].astype(np.float32)
    Wm = ii["Wm"].astype(np.float32)
    b_hh = ii["b_hh"].astype(np.float32)
    # one-hot-row weights: gi r/z parts (+b_ih+b_hh) in cols [0:602]; cols
    # [602:903] carry only b_hh's candidate part (inn itself is computed
    # separately via wpreI)
    wpreA = W_ih.T.copy()
    wpreA[:, 602:903] = 0.0
    wpreA[:NT, 0:602] += (b_ih + b_hh)[None, 0:602]
    wpreA[:NT, 602:903] += b_hh[None, 602:903]
    wpreI = W_ih.T[:, 602:903].copy()
    wpreI[:NT] += b_ih[None, 602:903]
    wpreB = np.zeros((19, 602), np.float32)
    wpreB[NT:19, 0:HID] = Wg[:, HID:HID + PP].T + bg[None, :]
    wpreB[NT:19, HID:602] = Wm[:, HID:HID + PP].T
    whhT = ii["W_hh"].astype(np.float32).T.copy()       # [301, 903]
    wgm = np.zeros((HID, 602), np.float32)
    wgm[:, 0:HID] = Wg[:, 0:HID].T
    wgm[:, HID:602] = Wm[:, 0:HID].T
    # h-side k2: rows 0:45 = W_hh^T tail, 45:64 zero pad, 64:83 one-hot
    whh2 = np.zeros((83, GI), np.float32)
    whh2[0:45] = whhT[256:301]
    whh2[64:83] = wpreA
    # gates combo: rows 0:19 = one-hot (pos+bg), rows 64:109 = wgm tail
    wBc109 = np.zeros((109, 602), np.float32)
    wBc109[0:19] = wpreB
    wBc109[64:109] = wgm[256:301]
    # FC lhsT rows: [0:256) = h dims 0:256 (two 128-chunks); tail chunk of 97
    # rows: 0:45 h-tail, 64:72 Hd, 96 biases (matches fcK2 on-device layout)
    wfcT1 = ii["W_fc1"].astype(np.float32).T   # [309, 56]
    wfcT2 = ii["W_fc2"].astype(np.float32).T
    wfc = np.zeros((353, OUT_W), np.float32)
    wfc[0:256, 0:56] = wfcT1[0:256]
    wfc[0:256, 56:112] = wfcT2[0:256]
    tail = np.zeros((97, OUT_W), np.float32)
    tail[0:45, 0:56] = wfcT1[256:301]
    tail[0:45, 56:112] = wfcT2[256:301]
    tail[64:72, 0:56] = wfcT1[301:309]
    tail[64:72, 56:112] = wfcT2[301:309]
    tail[96, 0:56] = ii["b_fc1"].astype(np.float32)
    tail[96, 56:112] = ii["b_fc2"].astype(np.float32)
    wfc[256:353] = tail
    wdf1 = np.zeros((33, 16), np.float32)
    wdf1[0:27] = ii["W_df1"].astype(np.float32).T
    wdf1[32] = ii["b_df1"].astype(np.float32)
    wdf2 = np.zeros((33, 8), np.float32)
    wdf2[0:16] = ii["W_df2"].astype(np.float32).T
    wdf2[32] = ii["b_df2"].astype(np.float32)

    # one mega weight block [128, 5176]
    wpk = np.zeros((128, 5176), np.float32)
    wpk[:, 0:903] = whhT[0:128]
    wpk[:, 903:1806] = whhT[128:256]
    wpk[0:83, 1806:2709] = whh2
    wpk[64:83, 2709:3010] = wpreI
    wpk[:, 3010:3612] = wgm[0:128]
    wpk[:, 3612:4214] = wgm[128:256]
    wpk[0:109, 4214:4816] = wBc109
    wpk[:, 4816:4928] = wfc[0:128]
    wpk[:, 4928:5040] = wfc[128:256]
    wpk[0:97, 5040:5152] = wfc[256:353]
    wpk[0:33, 5152:5168] = wdf1
    wpk[0:33, 5168:5176] = wdf2
    return {"wpack": np.ascontiguousarray(wpk).astype(np.float16)}


def shard_inputs(inputs):
    ii = {k: np.asarray(v) for k, v in inputs.items()}
    w = prepack(ii)
    types = ii["types"].astype(np.int32)
    paths = ii["paths"].astype(np.int32)
    adj = ii["adj_raw"].reshape(B, MAX_N * MAX_N).astype(np.int32)
    feats = ii["feats"].reshape(B, 3 * MAX_N).astype(np.float32).view(np.int32)
    packed = np.concatenate([types, paths, adj, feats], axis=1)
    maps = []
    for i in range(NCORES):
        sl = slice(i * BL, (i + 1) * BL)
        m = dict(packed=np.ascontiguousarray(packed[sl]), **w)
        maps.append(m)
    return maps


def get_nc():
    if "nc" not in _CACHE:
        _CACHE["nc"] = build_nc()
    return _CACHE["nc"]


def kernel(**inputs):
    nc = get_nc()
    maps = shard_inputs(inputs)
    res = run_bass_kernel_spmd(nc, maps, list(range(NCORES)))
    out = np.concatenate([res.results[i]["out"] for i in range(NCORES)], axis=0)
    return np.ascontiguousarray(out.astype(np.float32))
